# revision 1
# baseline (speedup 1.0000x reference)
"""Trainium2 Bass kernel for FNO1d (B=64, N=8192, W=64, MODES=16, 4 layers).

Pure data-parallel over batch: 8 elements per NeuronCore on 8 cores. Per core,
elements form 4 pairs (2x64 channels -> 128 partitions). rfft/irfft are
replaced by 16-mode DFT matmuls. All activations/weights are fp16 in SBUF
(accumulation in fp32 PSUM); biases are folded into matmuls (fc0 via a ones
row, layer cb via an extra ones-row of the inverse basis).
"""
import sys
import numpy as np

sys.path.insert(0, "/opt/trn_rl_repo")

import concourse.bacc as bacc
import concourse.mybir as mybir
import concourse.tile as tile
from concourse.bass_utils import run_bass_kernel_spmd

F16 = mybir.dt.float16
F32 = mybir.dt.float32
GELU = mybir.ActivationFunctionType.Gelu
COPY = mybir.ActivationFunctionType.Copy

B, N, W, MODES, L = 64, 8192, 64, 16, 4
NCORES = 8
E = B // NCORES          # 8 elems per core
NP = E // 2              # 4 pairs
K2 = 2 * MODES           # 32 interleaved (cos,sin) rows
CH = 512
GRP = 1024
NGRP = N // GRP          # 8
NA = N // 128            # 64 transpose chunks
SL = 2048                # transpose slice width
NSL = N // SL            # 4 slices
APS = SL // 128          # a-chunks per slice = 16

_cache = {}


def _build():
    if "nc" in _cache:
        return _cache["nc"]
    nc = bacc.Bacc("TRN2", target_bir_lowering=False, debug=False,
                   num_devices=NCORES)

    def din(name, shape, dt):
        return nc.dram_tensor(name, shape, dt, kind="ExternalInput").ap()

    xt_d = din("xt", [128, N], F16)      # rows 32p..32p+6 = (xA, tA, 1, xB, tB, 1)
    fb_d = din("fb", [128, NA, K2], F16)         # fb[p,a,kk] = Fb[128a+p, kk]
    ib_d = din("ib", [128, N], F16)              # rows 0:33 & 64:97 = [IB; ones]
    wm_d = din("wm", [64, L, MODES, 3, 64], F16)  # (i, l, k, {wr, wi, -wr}, o)
    cwt_d = din("cwt", [128, L, 128], F16)       # block-diag [cw.T 0; 0 cw.T]
    w03_d = din("w03", [2, 64], F16)             # (w0; w1) for spectral outer
    w1b_d = din("w1blk", [128, 128], F16)        # layer0 v: rank-3 block at rows 32p
    fc1w_d = din("fc1w", [128, 128], F16)        # both bands = fc1_w
    fc2w_d = din("fc2w", [128, 1], F16)
    cbr_d = din("cbrep", [128, L * 512], F16)    # rows 32/96: tile(cb[l], 8)
    fc1b_d = din("fc1b", [128, 1], F32)
    fc2b_d = din("fc2b", [128, 1], F32)
    idn_d = din("idn", [64, 64], F16)
    out_d = nc.dram_tensor("out", [E, N], F32, kind="ExternalOutput").ap()

    with tile.TileContext(nc) as tc:
        import contextlib
        with contextlib.ExitStack() as ctx:
            const = ctx.enter_context(tc.tile_pool(name="const", bufs=1))
            hpool = ctx.enter_context(tc.tile_pool(name="h", bufs=1))
            htp = ctx.enter_context(tc.tile_pool(name="htp", bufs=1))
            small = ctx.enter_context(tc.tile_pool(name="small", bufs=2))
            wmp = ctx.enter_context(tc.tile_pool(name="wmp", bufs=1))
            gpool = ctx.enter_context(tc.tile_pool(name="g", bufs=2))
            opool = ctx.enter_context(tc.tile_pool(name="o", bufs=2))
            ps = ctx.enter_context(tc.tile_pool(name="ps", bufs=1, space="PSUM"))
            psz = ctx.enter_context(tc.tile_pool(name="psz", bufs=2, space="PSUM"))

            # ---- constants ----
            fb = const.tile([128, NA * K2], F16, tag="fb")
            nc.sync.dma_start(out=fb[:], in_=fb_d.rearrange("p a k -> p (a k)"))
            ib = const.tile([128, N], F16, tag="ib")
            nc.sync.dma_start(out=ib[:], in_=ib_d)
            cwt = const.tile([128, L * 128], F16, tag="cwt")
            nc.sync.dma_start(out=cwt[:], in_=cwt_d.rearrange("p l o -> p (l o)"))
            w03 = const.tile([2, 64], F16, tag="w03")
            nc.sync.dma_start(out=w03[:], in_=w03_d)
            w1blk = const.tile([128, 128], F16, tag="w1blk")
            nc.sync.dma_start(out=w1blk[:], in_=w1b_d)
            fc1w = const.tile([128, 128], F16, tag="fc1w")
            nc.sync.dma_start(out=fc1w[:], in_=fc1w_d)
            fc2w = const.tile([128, 1], F16, tag="fc2w")
            nc.sync.dma_start(out=fc2w[:], in_=fc2w_d)
            cbrep = const.tile([128, L * 512], F16, tag="cbrep")
            nc.sync.dma_start(out=cbrep[:], in_=cbr_d)
            fc1b = const.tile([128, 1], F32, tag="fc1b")
            nc.sync.dma_start(out=fc1b[:], in_=fc1b_d)
            fc2b = const.tile([128, 1], F32, tag="fc2b")
            nc.sync.dma_start(out=fc2b[:], in_=fc2b_d)
            idn = const.tile([64, 64], F16, tag="idn")
            nc.sync.dma_start(out=idn[:], in_=idn_d)
            xt = const.tile([128, N], F16, tag="xt")
            nc.sync.dma_start(out=xt[:], in_=xt_d)

            # persistent per-pair activations [128, N] fp16
            h = [hpool.tile([128, N], F16, tag=f"h{p}", name=f"h{p}")
                 for p in range(NP)]


            # ---- FNO layers ----
            for l in range(L):
                wml = wmp.tile([64, MODES * 3 * 64], F16, tag="wml")
                nc.sync.dma_start(out=wml[:], in_=wm_d[:, l].rearrange("p b c d -> p (b c d)"))

                ftp = ps.tile([K2, 4 * 128], F32, tag="ft")
                if l == 0:
                    # spectral shortcut: ft(h0) from x/t spectra (h0 never built)
                    xtt = htp.tile([128, NA * 128], F16, tag="hta", name="xtt")
                    xtt4 = xtt[:].rearrange("p (a c) -> p a c", a=NA)
                    for p in range(NP):
                        nc.sync.dma_start(out=xtt4[:, :, 32 * p:32 * p + 32],
                                          in_=xt[32 * p:32 * p + 32, :],
                                          transpose=True)
                    fxp = ps.tile([K2, 256], F32, tag="mm")
                    for a in range(NA):
                        nc.tensor.matmul(fxp[:, 0:128], fb[:, a * K2:(a + 1) * K2],
                                         xtt[:, a * 128:(a + 1) * 128],
                                         start=(a == 0), stop=(a == NA - 1))
                    fxs = small.tile([K2, 128], F16, tag="fxs")
                    nc.vector.tensor_copy(fxs[:], fxp[:, 0:128])
                    # per elem: transpose (x,t) spectra -> [2, 32]
                    ftx = ps.tile([2, 256], F16, tag="mtops")
                    for p in range(NP):
                        for e2 in range(2):
                            e = 2 * p + e2
                            cc = 32 * p + 3 * e2
                            nc.tensor.transpose(ftx[0:2, 32 * e:32 * e + 32],
                                                fxs[:, cc:cc + 2], idn[0:K2, 0:K2])
                    fxt = small.tile([2, 256], F16, tag="fxt")
                    nc.vector.tensor_copy(fxt[:], ftx[:])
                    # outer product: ftp[:, 64 e:64 e+64] = fxt_e.T @ (w0; w1)
                    for e in range(8):
                        nc.tensor.matmul(ftp[:, 64 * e:64 * e + 64],
                                         fxt[0:2, 32 * e:32 * e + 32],
                                         w03[0:2, 0:64], start=True, stop=True)
                else:
                    # transpose h slices into hta (cols = (a, pair, c)), then FT
                    hta = htp.tile([128, NA * 512], F16, tag="hta", name=f"hta{l}")
                    hta4 = hta[:].rearrange("p (a e c) -> p a e c", a=NA, e=NP)
                    for p in range(NP):
                        for q in range(NSL):
                            nc.sync.dma_start(
                                out=hta4[:, q * APS:(q + 1) * APS, p, :],
                                in_=h[p][:, q * SL:(q + 1) * SL], transpose=True)
                    for a in range(NA):
                        nc.tensor.matmul(ftp[:], fb[:, a * K2:(a + 1) * K2],
                                         hta[:, a * 512:(a + 1) * 512],
                                         start=(a == 0), stop=(a == NA - 1))
                ftsb = small.tile([K2, 512], F16, tag="ftsb")
                nc.vector.tensor_copy(ftsb[:], ftp[:])

                # FTI[i, 32 e + kk] via 2 stream transposes (i-halves)
                fti = small.tile([64, 256], F16, tag="fti")
                ftsb3 = ftsb[:].rearrange("p (e c) -> p e c", e=8)
                for hh in range(2):
                    nc.vector.transpose(fti[32 * hh:32 * hh + 32, :],
                                        ftsb3[:, :, 32 * hh:32 * hh + 32])

                # mode mix -> mmp[o, 8*(2k+s') + e]
                mmp = ps.tile([64, 256], F32, tag="mm")
                for k in range(MODES):
                    wb = (k * 3) * 64
                    wr = wml[:, wb:wb + 64]
                    wi = wml[:, wb + 64:wb + 128]
                    nwr = wml[:, wb + 128:wb + 192]
                    ftc = fti[:, 2 * k::K2]
                    fts = fti[:, 2 * k + 1::K2]
                    nc.tensor.matmul(mmp[:, 16 * k:16 * k + 8], wr, ftc,
                                     start=True, stop=False)
                    nc.tensor.matmul(mmp[:, 16 * k:16 * k + 8], wi, fts,
                                     start=False, stop=True)
                    nc.tensor.matmul(mmp[:, 16 * k + 8:16 * k + 16], wi, ftc,
                                     start=True, stop=False)
                    nc.tensor.matmul(mmp[:, 16 * k + 8:16 * k + 16], nwr, fts,
                                     start=False, stop=True)
                mmsb = small.tile([64, 256], F16, tag="mmsb")
                nc.vector.tensor_copy(mmsb[:], mmp[:])

                # MT[kk, 64 e + o] via 8 PE transposes; row 32 = cb bias
                mtp = ps.tile([128, 512], F16, tag="mtops")
                for e in range(8):
                    src = mmsb[:, e::8]   # [64 o, 32 kk]
                    nc.tensor.transpose(mtp[0:32, 64 * e:64 * e + 64], src, idn[:])
                mt = small.tile([64, 512], F16, tag="mtsb")
                nc.vector.tensor_copy(mt[0:32, :], mtp[0:32, :])
                nc.vector.tensor_copy(mt[32:33, :], cbrep[32:33, l * 512:(l + 1) * 512])

                # inverse FT (K=33 incl. bias row) + pointwise (block-diag) + act
                act = GELU if l < L - 1 else COPY
                for p in range(NP):
                    for g in range(NGRP):
                        zp = psz.tile([128, GRP], F32, tag="z")
                        for c2 in range(GRP // CH):
                            c0 = g * GRP + c2 * CH
                            sl = slice(c2 * CH, (c2 + 1) * CH)
                            if l == 0:
                                nc.tensor.matmul(zp[:, sl], w1blk[32 * p:32 * p + 6, :],
                                                 xt[32 * p:32 * p + 6, c0:c0 + CH],
                                                 start=True, stop=False,
                                                 tile_position=(32 * p, 0))
                            else:
                                nc.tensor.matmul(zp[:, sl], cwt[:, l * 128:(l + 1) * 128],
                                                 h[p][:, c0:c0 + CH], start=True, stop=False)
                        for c2 in range(GRP // CH):
                            c0 = g * GRP + c2 * CH
                            sl = slice(c2 * CH, (c2 + 1) * CH)
                            nc.tensor.matmul(zp[:, sl],
                                             mt[0:33, 128 * p:128 * p + 128],
                                             ib[0:33, c0:c0 + CH], start=False, stop=True)
                        nc.scalar.activation(h[p][:, g * GRP:(g + 1) * GRP], zp[:], act)

            # ---- fc1 -> gelu -> fc2 -> out ----
            for p in range(NP):
                for g in range(NGRP):
                    op = ps.tile([128, GRP], F32, tag="mtops")
                    gsbs = []
                    for e2 in range(2):
                        gp = psz.tile([128, GRP], F32, tag="z")
                        for c2 in range(GRP // CH):
                            c0 = g * GRP + c2 * CH
                            nc.tensor.matmul(gp[:, c2 * CH:(c2 + 1) * CH],
                                             fc1w[64 * e2:64 * e2 + 64, :],
                                             h[p][64 * e2:64 * e2 + 64, c0:c0 + CH],
                                             start=True, stop=True)
                        gsb = gpool.tile([128, GRP], F16, tag=f"gsb{e2}", name=f"gsb{e2}")
                        nc.scalar.activation(gsb[:], gp[:], GELU, bias=fc1b[:])
                        gsbs.append(gsb)
                    for c2 in range(GRP // CH):
                        sl = slice(c2 * CH, (c2 + 1) * CH)
                        for e2 in range(2):
                            nc.tensor.matmul(op[32 * e2:32 * e2 + 1, sl], fc2w[:],
                                             gsbs[e2][:, sl], start=True, stop=True,
                                             tile_position=(0, 32 * e2))
                    osb = opool.tile([64, GRP], F32, tag="osb")
                    nc.vector.tensor_scalar_add(osb[:], op[0:64, :], fc2b[0:64, :])
                    for e2 in range(2):
                        e = 2 * p + e2
                        nc.sync.dma_start(out=out_d[e:e + 1, g * GRP:(g + 1) * GRP],
                                          in_=osb[32 * e2:32 * e2 + 1, :])

    nc.compile()
    _cache["nc"] = nc
    return nc


def _consts(fc0_w, fc0_b, sw_r, sw_i, cw, cb, fc1_w, fc1_b, fc2_w, fc2_b):
    f16 = np.float16
    n = np.arange(N)
    k = np.arange(MODES)
    ang = 2.0 * np.pi * np.outer(n, k) / N            # [N, MODES]
    Fb = np.empty((N, K2), np.float32)
    Fb[:, 0::2] = np.cos(ang)
    Fb[:, 1::2] = np.sin(ang)
    fb = np.ascontiguousarray(
        Fb.reshape(NA, 128, K2).transpose(1, 0, 2)).astype(f16)

    alpha = np.where(k == 0, 1.0, 2.0)
    IBr = np.empty((K2, N), np.float32)
    IBr[0::2, :] = (alpha[:, None] * np.cos(ang.T)) / N
    IBr[1::2, :] = -2.0 * np.sin(ang.T) / N
    ib = np.zeros((128, N), np.float32)
    ib[0:K2] = IBr
    ib[K2, :] = 1.0
    ib = ib.astype(f16)

    wm = np.empty((64, L, MODES, 3, 64), np.float32)
    for l in range(L):
        for kk in range(MODES):
            wm[:, l, kk, 0, :] = sw_r[l, :, :, kk]
            wm[:, l, kk, 1, :] = sw_i[l, :, :, kk]
            wm[:, l, kk, 2, :] = -sw_r[l, :, :, kk]
    wm = wm.astype(f16)

    cwt = np.zeros((128, L, 128), np.float32)
    for l in range(L):
        cwt[0:64, l, 0:64] = cw[l].T
        cwt[64:128, l, 64:128] = cw[l].T

    cbrep = np.zeros((128, L * 512), np.float32)
    for l in range(L):
        cbl = cb[l].astype(np.float32).copy()
        if l == 0:
            # DC correction: spectral path drops sum_n(b) term; fold into bias
            cbl = cbl + sw_r[0, :, :, 0].T @ fc0_b
        cbrep[K2, l * 512:(l + 1) * 512] = np.tile(cbl, 8)

    w03 = fc0_w.astype(np.float32)                      # [2, 64]
    u = np.stack([cw[0] @ fc0_w[0], cw[0] @ fc0_w[1], cw[0] @ fc0_b], axis=0)
    w1blk = np.zeros((128, 128), np.float32)
    for p in range(NP):
        w1blk[32 * p:32 * p + 3, 0:64] = u
        w1blk[32 * p + 3:32 * p + 6, 64:128] = u
    return dict(
        fb=fb, ib=ib, wm=wm,
        cwt=cwt.astype(f16), cbrep=cbrep.astype(f16), w03=w03.astype(f16),
        w1blk=w1blk.astype(f16),
        fc1w=np.concatenate([fc1_w, fc1_w], axis=0).astype(f16),
        fc2w=fc2_w.astype(f16),
        fc1b=fc1_b.astype(np.float32).reshape(128, 1),
        fc2b=np.full((128, 1), np.float32(np.asarray(fc2_b).reshape(-1)[0])),
        idn=np.eye(64, dtype=f16),
    )


def kernel(x, t, fc0_w, fc0_b, sw_r, sw_i, cw, cb, fc1_w, fc1_b, fc2_w, fc2_b,
           _trace=False, _tmpdir=None):
    nc = _build()
    consts = _consts(np.asarray(fc0_w), np.asarray(fc0_b), np.asarray(sw_r),
                     np.asarray(sw_i), np.asarray(cw), np.asarray(cb),
                     np.asarray(fc1_w), np.asarray(fc1_b), np.asarray(fc2_w),
                     np.asarray(fc2_b))
    x = np.asarray(x, np.float32).reshape(B, N).astype(np.float16)
    t = np.asarray(t, np.float32).reshape(B, N).astype(np.float16)
    in_maps = []
    for c in range(NCORES):
        m = dict(consts)
        xt = np.ones((128, N), np.float16)
        for p in range(NP):
            eA, eB = c * E + 2 * p, c * E + 2 * p + 1
            xt[32 * p + 0] = x[eA]
            xt[32 * p + 1] = t[eA]
            xt[32 * p + 3] = x[eB]
            xt[32 * p + 4] = t[eB]
        m["xt"] = xt
        in_maps.append(m)
    res = run_bass_kernel_spmd(nc, in_maps, list(range(NCORES)),
                               trace=_trace, tmpdir=_tmpdir)
    out = np.concatenate([res.results[c]["out"] for c in range(NCORES)], axis=0)
    kernel.last_result = res
    return out.reshape(B, N, 1).astype(np.float32)



# revision 2
# speedup vs baseline: 1.1826x; 1.1826x over previous
"""Trainium2 Bass kernel for FNO1d (B=64, N=8192, W=64, MODES=16, 4 layers) — v2.

Data-parallel over batch: 8 elements/core, 4 pairs (2x64ch = 128 partitions).
rfft/irfft replaced by 16-mode DFT matmuls.

v2 optimizations over baseline:
- pointwise conv packed 4-elements-per-PE-pass via tile_position quadrants
  (off-diagonal quadrants swap element slots of the 2nd pair of each duo;
  compensated in the inverse-FT weight placement and final output mapping)
- inverse-FT as K=32 row-tiled matmuls, 2 pairs concurrent
- forward FT 4-way col-tiled (4 DFT chunks concurrently), partial sums
  summed with a stacked-identity matmul
- layer bias cb via ACT bias (fp32); layer-3 bias folded into fc1 bias
- fc1 2-way row-tiled; fc2 4-way col-tiled M=1; fc2 bias added on host
- mode-mix transposes on DVE (32x32 stream transposes) directly into SBUF
- fc head interleaved with layer 3; transposes split across sync/scalar DMA
  queues; FT accumulation runs in windows during the producing layer
"""
import sys
import numpy as np

sys.path.insert(0, "/opt/trn_rl_repo")

import concourse.bacc as bacc
import concourse.mybir as mybir
import concourse.tile as tile
from concourse.bass_utils import run_bass_kernel_spmd

F16 = mybir.dt.float16
F32 = mybir.dt.float32
GELU = mybir.ActivationFunctionType.Gelu

B, W, MODES, L = 64, 64, 16, 4
NCORES = 8
E = B // NCORES          # 8 elems per core
NP = E // 2              # 4 pairs
K2 = 2 * MODES           # 32 interleaved (cos,sin) rows

_cache = {}


def _build(N=8192):
    key = ("nc", N)
    if key in _cache:
        return _cache[key]
    NA = N // 128            # DFT chunks
    NGRP = N // 1024         # 1024-col groups per pair
    NW = NGRP // 2           # transpose windows (2048 cols each)
    APW = NA // NW           # a-chunks per window (16)
    assert APW == 16

    nc = bacc.Bacc("TRN2", target_bir_lowering=False, debug=False,
                   num_devices=NCORES)

    def din(name, shape, dt):
        return nc.dram_tensor(name, shape, dt, kind="ExternalInput").ap()

    xt_d = din("xt", [128, N], F16)       # rows 32p+(0..5) = (xA,tA,1,xB,tB,1)
    fb_d = din("fb", [128, NA * K2], F16)     # fb[p, a*32+kk] = Fb[128a+p, kk]
    ib_d = din("ib", [128, N], F16)           # rows 32p..32p+32 = IBr (x4)
    wm_d = din("wm", [64, L, MODES, 3, 64], F16)   # (i, l, k, {wr,wi,-wr}, o)
    cwt_d = din("cwt", [128, L * 128], F16)   # cw[l].T in all 4 quadrants
    w03_d = din("w03", [2, 64], F16)          # (w0; w1)
    w1b_d = din("w1blk", [128, 128], F16)     # layer0 v: rank-3 blocks at 32p
    fc1w_d = din("fc1w", [128, 128], F16)     # fc1_w duplicated both halves
    fc2w_d = din("fc2w", [128, 1], F16)
    cbb_d = din("cbb", [128, 4], F32)         # per-layer ACT bias (l=3 unused)
    fc1b_d = din("fc1b", [128, 1], F32)       # fc1_b + fc1_w.T @ cb3
    i4_d = din("i4", [128, K2], F16)          # 4 stacked 32-identities
    idn_d = din("idn", [64, 64], F16)
    out_d = nc.dram_tensor("out", [E, N], F32, kind="ExternalOutput").ap()

    with tile.TileContext(nc) as tc:
        import contextlib
        with contextlib.ExitStack() as ctx:
            const = ctx.enter_context(tc.tile_pool(name="const", bufs=1))
            hpool = ctx.enter_context(tc.tile_pool(name="h", bufs=1))
            htp = ctx.enter_context(tc.tile_pool(name="htp", bufs=1))
            small = ctx.enter_context(tc.tile_pool(name="small", bufs=2))
            wmp = ctx.enter_context(tc.tile_pool(name="wmp", bufs=2))
            gpool = ctx.enter_context(tc.tile_pool(name="g", bufs=3))
            opool = ctx.enter_context(tc.tile_pool(name="o", bufs=2))
            ps = ctx.enter_context(tc.tile_pool(name="ps", bufs=1, space="PSUM"))

            # ---- constants (xt first so transposes can start early) ----
            xt = const.tile([128, N], F16, tag="xt")
            nc.sync.dma_start(out=xt[:], in_=xt_d)
            fb = const.tile([128, NA * K2], F16, tag="fb")
            nc.sync.dma_start(out=fb[:], in_=fb_d)

            # persistent activations, one big tile [128, 4 pairs * N]
            h_all = hpool.tile([128, NP * N], F16, tag="h_all")
            # transposed view for FT: cols = (a, pair, slot*64+c)
            hta = htp.tile([128, NA * 512], F16, tag="hta")
            hta4 = hta[:].rearrange("p (a e c) -> p a e c", a=NA, e=NP)

            # layer-0 shortcut: transpose xt rows into the start of hta
            # (baseline xtt layout: col = a*128 + 32p + q, contiguous chunks)
            xtt = hta[:, 0:NA * 128].rearrange("p (a c) -> p a c", a=NA)
            for p in range(NP):
                eng = nc.sync if p % 2 == 0 else nc.scalar
                eng.dma_start(out=xtt[:, :, 32 * p:32 * p + K2],
                              in_=xt[32 * p:32 * p + K2, :], transpose=True)

            ib = const.tile([128, N], F16, tag="ib")
            nc.sync.dma_start(out=ib[:], in_=ib_d)
            cwt = const.tile([128, L * 128], F16, tag="cwt")
            nc.sync.dma_start(out=cwt[:], in_=cwt_d)
            w03 = const.tile([2, 64], F16, tag="w03")
            nc.sync.dma_start(out=w03[:], in_=w03_d)
            w1blk = const.tile([128, 128], F16, tag="w1blk")
            nc.sync.dma_start(out=w1blk[:], in_=w1b_d)
            fc1w = const.tile([128, 128], F16, tag="fc1w")
            nc.sync.dma_start(out=fc1w[:], in_=fc1w_d)
            fc2w = const.tile([128, 1], F16, tag="fc2w")
            nc.sync.dma_start(out=fc2w[:], in_=fc2w_d)
            cbb = const.tile([128, 4], F32, tag="cbb")
            nc.sync.dma_start(out=cbb[:], in_=cbb_d)
            fc1b = const.tile([128, 1], F32, tag="fc1b")
            nc.sync.dma_start(out=fc1b[:], in_=fc1b_d)
            i4 = const.tile([128, K2], F16, tag="i4")
            nc.sync.dma_start(out=i4[:], in_=i4_d)
            idn = const.tile([64, 64], F16, tag="idn")
            nc.sync.dma_start(out=idn[:], in_=idn_d)

            wml = [None] * L
            wml[0] = wmp.tile([64, MODES * 3 * 64], F16, tag="wml", name="wml0")
            nc.sync.dma_start(out=wml[0][:],
                              in_=wm_d[:, 0].rearrange("p b c d -> p (b c d)"))

            def mix_chain(l, ftsb):
                """ftsb [32, 512] f16 (kk x (pair,slot,c)) -> mt4 [128,128] f16.

                mt4 rows 32p..32p+32 = kk, cols 64*tslot..+64 = o for pair p.
                tslot crossing for pairs 1,3 on quadrant layers (l>=1).
                """
                fti = small.tile([64, 256], F16, tag="fti")
                for hh in range(2):
                    for e in range(8):
                        nc.vector.transpose(
                            fti[32 * hh:32 * hh + 32, 32 * e:32 * e + 32],
                            ftsb[0:K2, 64 * e + 32 * hh:64 * e + 32 * hh + 32])
                mmp = ps.tile([64, 256], F32, tag="aux", name=f"mm{l}")
                for k in range(MODES):
                    wb = (k * 3) * 64
                    wr = wml[l][:, wb:wb + 64]
                    wi = wml[l][:, wb + 64:wb + 128]
                    nwr = wml[l][:, wb + 128:wb + 192]
                    ftc = fti[:, 2 * k::K2]
                    fts = fti[:, 2 * k + 1::K2]
                    nc.tensor.matmul(mmp[:, 16 * k:16 * k + 8], wr, ftc,
                                     start=True, stop=False)
                    nc.tensor.matmul(mmp[:, 16 * k:16 * k + 8], wi, fts,
                                     start=False, stop=True)
                    nc.tensor.matmul(mmp[:, 16 * k + 8:16 * k + 16], wi, ftc,
                                     start=True, stop=False)
                    nc.tensor.matmul(mmp[:, 16 * k + 8:16 * k + 16], nwr, fts,
                                     start=False, stop=True)
                mmsb = small.tile([64, 256], F16, tag="mmsb")
                nc.vector.tensor_copy(mmsb[:], mmp[:])
                # mmsb[o, 8*kk + eidx]; build mt4 via DVE 32x32 transposes
                mt4 = small.tile([128, 128], F16, tag="mt4")
                for p in range(NP):
                    for s in range(2):
                        eidx = 2 * p + s
                        tslot = (1 - s) if (l >= 1 and p % 2 == 1) else s
                        for oh in range(2):
                            nc.vector.transpose(
                                mt4[32 * p:32 * p + 32,
                                    64 * tslot + 32 * oh:64 * tslot + 32 * oh + 32],
                                mmsb[32 * oh:32 * oh + 32, eidx::8])
                return mt4

            # ---------------- layer 0 head: ft from x/t spectra -------------
            fxp = ps.tile([128, 128], F32, tag="aux", name="fxp")
            nc.vector.memset(fxp[:], 0)
            for t in range(NA // 4):
                for j in range(4):
                    a = 4 * t + j
                    nc.tensor.matmul(fxp[32 * j:32 * j + 32, :],
                                     fb[:, a * K2:(a + 1) * K2],
                                     hta[:, a * 128:(a + 1) * 128],
                                     start=False, stop=False,
                                     skip_group_check=True,
                                     tile_position=(0, 32 * j))
            fxsb = small.tile([128, 128], F16, tag="fxsb")
            nc.vector.tensor_copy(fxsb[:], fxp[:])
            fxps = ps.tile([K2, 128], F32, tag="aux", name="fxps")
            nc.tensor.matmul(fxps[:], i4[:], fxsb[:], start=True, stop=True)
            fxs2 = small.tile([K2, 128], F16, tag="fxsb2")
            nc.vector.tensor_copy(fxs2[:], fxps[:])
            # per slot-elem transpose of (x,t) spectra -> [2, 32]
            ftxp = ps.tile([2, 256], F16, tag="aux", name="ftxp")
            for p in range(NP):
                for s in range(2):
                    cc = 32 * p + 3 * s
                    nc.tensor.transpose(ftxp[0:2, 32 * (2 * p + s):32 * (2 * p + s) + 32],
                                        fxs2[:, cc:cc + 2], idn[0:K2, 0:K2])
            fxt = small.tile([2, 256], F16, tag="fxt")
            nc.vector.tensor_copy(fxt[:], ftxp[:])
            ft0 = ps.tile([K2, 512], F32, tag="aux", name="ft0")
            for e in range(8):
                nc.tensor.matmul(ft0[:, 64 * e:64 * e + 64],
                                 fxt[0:2, 32 * e:32 * e + 32],
                                 w03[0:2, 0:64], start=True, stop=True)
            ftsb0 = small.tile([K2, 512], F16, tag="ftsb")
            nc.vector.tensor_copy(ftsb0[:], ft0[:])

            # ---------------- FNO layers ----------------
            for l in range(L):
                if l > 0:
                    ftpsb = small.tile([128, 512], F16, tag="ftpsb")
                    nc.vector.tensor_copy(ftpsb[:], ftp[:])  # noqa: F821
                    ftt = ps.tile([K2, 512], F32, tag="aux", name=f"ftt{l}")
                    nc.tensor.matmul(ftt[:], i4[:], ftpsb[:], start=True, stop=True)
                    ftsb = small.tile([K2, 512], F16, tag="ftsb")
                    nc.vector.tensor_copy(ftsb[:], ftt[:])
                else:
                    ftsb = ftsb0
                mt4 = mix_chain(l, ftsb)
                if l + 1 < L:
                    wml[l + 1] = wmp.tile([64, MODES * 3 * 64], F16, tag="wml",
                                          name=f"wml{l + 1}")
                    nc.sync.dma_start(
                        out=wml[l + 1][:],
                        in_=wm_d[:, l + 1].rearrange("p b c d -> p (b c d)"))
                if l < L - 1:
                    ftp = ps.tile([128, 512], F32, tag="ftp", name=f"ftp{l}")
                    nc.vector.memset(ftp[:], 0)

                def ft_window(w, half=None):
                    # transposes for window w (reads h_all written this layer);
                    # half=0/1 transposes only 1024 cols (8 a-chunks) for the
                    # fine-grained layer tail
                    a0 = w * APW if half in (None, 0) else w * APW + APW // 2
                    na = APW if half is None else APW // 2
                    for p in range(NP):
                        eng = nc.sync if p % 2 == 0 else nc.scalar
                        eng.dma_start(
                            out=hta4[:, a0:a0 + na, p, :],
                            in_=h_all[:, p * N + a0 * 128:p * N + (a0 + na) * 128],
                            transpose=True)

                def ft_burst(w, first=False, last=False, half=None):
                    t0 = 0 if half in (None, 0) else APW // 8
                    tn = APW // 4 if half is None else APW // 8
                    for t in range(t0, t0 + tn):
                        for j in range(4):
                            a = w * APW + 4 * t + j
                            nc.tensor.matmul(
                                ftp[32 * j:32 * j + 32, :],
                                fb[:, a * K2:(a + 1) * K2],
                                hta[:, a * 512:(a + 1) * 512],
                                start=False, stop=False,
                                skip_group_check=True,
                                tile_position=(0, 32 * j))

                act_l = cbb[:, l:l + 1]
                bursted = set()
                for g in range(NGRP):
                    for duo in range(2):
                        p0, p1 = 2 * duo, 2 * duo + 1
                        zp0 = gpsum_tile(ps, f"zp_l{l}g{g}d{duo}a")
                        zp1 = gpsum_tile(ps, f"zp_l{l}g{g}d{duo}b")
                        SL = (slice(0, 512), slice(512, 1024))
                        CO = (g * 1024, g * 1024 + 512)
                        # inverse FT first (start=True clears each bank);
                        # c2-runs back-to-back reuse the loaded weights
                        for p, zp in ((p0, zp0), (p1, zp1)):
                            for c2 in range(2):
                                nc.tensor.matmul(
                                    zp[:, SL[c2]], mt4[32 * p:32 * p + 32, :],
                                    ib[32 * p:32 * p + 32, CO[c2]:CO[c2] + 512],
                                    start=True, stop=False, skip_group_check=True,
                                    tile_position=(32 * p, 0))
                        if l == 0:
                            # layer-0 v from xt, rank-3 per pair, row-tiled
                            for p, zp in ((p0, zp0), (p1, zp1)):
                                for c2 in range(2):
                                    nc.tensor.matmul(
                                        zp[:, SL[c2]], w1blk[32 * p:32 * p + 6, :],
                                        xt[32 * p:32 * p + 6, CO[c2]:CO[c2] + 512],
                                        start=False, stop=True,
                                        skip_group_check=True,
                                        tile_position=(32 * p, 0))
                        else:
                            lb = l * 128
                            quads = (
                                (zp0, slice(0, 64), cwt[0:64, lb:lb + 64],
                                 p0, slice(0, 64), (0, 0), False),
                                (zp1, slice(64, 128), cwt[0:64, lb + 64:lb + 128],
                                 p1, slice(0, 64), (0, 64), False),
                                (zp0, slice(64, 128), cwt[64:128, lb + 64:lb + 128],
                                 p0, slice(64, 128), (64, 64), True),
                                (zp1, slice(0, 64), cwt[64:128, lb:lb + 64],
                                 p1, slice(64, 128), (64, 0), True),
                            )
                            for zp, osl, cwq, hp, hsl, tpos, stp in quads:
                                for c2 in range(2):
                                    nc.tensor.matmul(
                                        zp[osl, SL[c2]], cwq,
                                        h_all[hsl, hp * N + CO[c2]:hp * N + CO[c2] + 512],
                                        start=False, stop=(stp and c2 == 1),
                                        skip_group_check=True,
                                        tile_position=tpos)
                        for p, zp in ((p0, zp0), (p1, zp1)):
                            dst = h_all[:, p * N + g * 1024:p * N + (g + 1) * 1024]
                            if l < L - 1:
                                nc.scalar.activation(dst, zp[:], GELU, bias=act_l)
                            else:
                                nc.vector.tensor_copy(dst, zp[:])
                    if l < L - 1:
                        # issue transposes as soon as a window's groups done;
                        # the last window goes in 1024-col halves to shrink
                        # the layer tail
                        if g == NGRP - 2:
                            ft_window(NW - 1, half=0)
                        elif g == NGRP - 1:
                            ft_window(NW - 1, half=1)
                            ft_burst(NW - 1, half=0)
                        elif g % 2 == 1:
                            ft_window(g // 2)
                        # FT bursts trail the transposes by ~2 groups
                        if g >= 3 and g % 2 == 1 and (g - 3) // 2 < NW - 1:
                            w = (g - 3) // 2
                            ft_burst(w)
                            bursted.add(w)
                    if l == L - 1:
                        # interleave fc head chunk g (reads h_all groups <= g)
                        fc_chunk(nc, g, h_all, fc1w, fc2w, fc1b, out_d,
                                 gpool, opool, ps, N)
                if l < L - 1:
                    for w in range(NW - 1):
                        if w not in bursted:
                            ft_burst(w)
                    ft_burst(NW - 1, half=1)

    nc.compile()
    _cache[key] = nc
    return nc


def gpsum_tile(ps, name):
    return ps.tile([128, 1024], F32, tag="zp", name=name, bufs=3)


def fc_chunk(nc, g, h_all, fc1w, fc2w, fc1b, out_d, gpool, opool, ps, N):
    """fc1 -> gelu -> fc2 for 1024-col chunk g, all pairs."""
    for p in range(NP):
        gsb = []
        for j in range(2):
            gp = gpsum_tile(ps, f"gp_g{g}p{p}j{j}")
            for c2 in range(2):
                cols = g * 1024 + c2 * 512
                nc.tensor.matmul(
                    gp[:, c2 * 512:(c2 + 1) * 512],
                    fc1w[64 * j:64 * j + 64, :],
                    h_all[64 * j:64 * j + 64, p * N + cols:p * N + cols + 512],
                    start=True, stop=True, tile_position=(64 * j, 0))
            gs = gpool.tile([128, 1024], F16, tag=f"gsb{j}", name=f"gsb{g}{p}{j}")
            nc.scalar.activation(gs[:], gp[:], GELU, bias=fc1b[:])
            gsb.append(gs)
        op = ps.tile([128, 512], F32, tag="aux", name=f"op{g}{p}")
        nc.vector.memset(op[:], 0)
        for j in range(2):
            for c2 in range(2):
                r = 32 * (2 * j + c2)
                nc.tensor.matmul(op[r:r + 1, :], fc2w[:],
                                 gsb[j][:, c2 * 512:(c2 + 1) * 512],
                                 start=True, stop=True, tile_position=(0, r))
        osb = opool.tile([128, 512], F32, tag="osb", name=f"osb{g}{p}")
        nc.vector.tensor_copy(osb[:], op[:])
        for j in range(2):
            # pairs 1,3 end with slots swapped (odd number of quadrant layers)
            erow = 2 * p + (1 - j if p % 2 == 1 else j)
            for c2 in range(2):
                r = 32 * (2 * j + c2)
                cols = g * 1024 + c2 * 512
                nc.sync.dma_start(out=out_d[erow:erow + 1, cols:cols + 512],
                                  in_=osb[r:r + 1, :])


def _consts(N, fc0_w, fc0_b, sw_r, sw_i, cw, cb, fc1_w, fc1_b, fc2_w, fc2_b):
    f16 = np.float16
    NA = N // 128
    n = np.arange(N)
    k = np.arange(MODES)
    ang = 2.0 * np.pi * np.outer(n, k) / N            # [N, MODES]
    Fb = np.empty((N, K2), np.float32)
    Fb[:, 0::2] = np.cos(ang)
    Fb[:, 1::2] = np.sin(ang)
    fb = np.ascontiguousarray(
        Fb.reshape(NA, 128, K2).transpose(1, 0, 2)).reshape(128, NA * K2).astype(f16)

    alpha = np.where(k == 0, 1.0, 2.0)
    IBr = np.empty((K2, N), np.float32)
    IBr[0::2, :] = (alpha[:, None] * np.cos(ang.T)) / N
    IBr[1::2, :] = -2.0 * np.sin(ang.T) / N
    ib = np.zeros((128, N), np.float32)
    for p in range(NP):
        ib[32 * p:32 * p + K2] = IBr
    ib = ib.astype(f16)

    wm = np.empty((64, L, MODES, 3, 64), np.float32)
    for l in range(L):
        for kk in range(MODES):
            wm[:, l, kk, 0, :] = sw_r[l, :, :, kk]
            wm[:, l, kk, 1, :] = sw_i[l, :, :, kk]
            wm[:, l, kk, 2, :] = -sw_r[l, :, :, kk]
    wm = wm.astype(f16)

    cwt = np.zeros((128, L * 128), np.float32)
    for l in range(L):
        for qr in range(2):
            for qc in range(2):
                cwt[64 * qr:64 * qr + 64, l * 128 + 64 * qc:l * 128 + 64 * qc + 64] = cw[l].T

    cbb = np.zeros((128, 4), np.float32)
    for l in range(L - 1):
        cbl = cb[l].astype(np.float32).copy()
        if l == 0:
            cbl = cbl + sw_r[0, :, :, 0].T @ fc0_b
        cbb[0:64, l] = cbl
        cbb[64:128, l] = cbl

    w03 = fc0_w.astype(np.float32)                      # [2, 64]
    u = np.stack([cw[0] @ fc0_w[0], cw[0] @ fc0_w[1], cw[0] @ fc0_b], axis=0)
    w1blk = np.zeros((128, 128), np.float32)
    for p in range(NP):
        w1blk[32 * p:32 * p + 3, 0:64] = u
        w1blk[32 * p + 3:32 * p + 6, 64:128] = u

    fc1bp = fc1_b.astype(np.float32) + fc1_w.astype(np.float32).T @ cb[L - 1].astype(np.float32)

    i4c = np.zeros((128, K2), np.float32)
    for j in range(4):
        i4c[32 * j:32 * j + K2, :] = np.eye(K2)

    return dict(
        fb=fb, ib=ib, wm=wm,
        cwt=cwt.astype(f16), cbb=cbb, w03=w03.astype(f16),
        w1blk=w1blk.astype(f16),
        fc1w=np.concatenate([fc1_w, fc1_w], axis=0).astype(f16),
        fc2w=fc2_w.astype(f16),
        fc1b=fc1bp.reshape(128, 1),
        i4=i4c.astype(f16),
        idn=np.eye(64, dtype=f16),
    )


def make_xt(x, t, core, N):
    xt = np.ones((128, N), np.float16)
    for p in range(NP):
        eA, eB = core * E + 2 * p, core * E + 2 * p + 1
        xt[32 * p + 0] = x[eA]
        xt[32 * p + 1] = t[eA]
        xt[32 * p + 3] = x[eB]
        xt[32 * p + 4] = t[eB]
    return xt


def kernel(x, t, fc0_w, fc0_b, sw_r, sw_i, cw, cb, fc1_w, fc1_b, fc2_w, fc2_b,
           _trace=False, _tmpdir=None):
    N = np.asarray(x).shape[1]
    nc = _build(N)
    consts = _consts(N, np.asarray(fc0_w), np.asarray(fc0_b), np.asarray(sw_r),
                     np.asarray(sw_i), np.asarray(cw), np.asarray(cb),
                     np.asarray(fc1_w), np.asarray(fc1_b), np.asarray(fc2_w),
                     np.asarray(fc2_b))
    x = np.asarray(x, np.float32).reshape(-1, N).astype(np.float16)
    t = np.asarray(t, np.float32).reshape(-1, N).astype(np.float16)
    in_maps = []
    for c in range(NCORES):
        m = dict(consts)
        m["xt"] = make_xt(x, t, c, N)
        in_maps.append(m)
    res = run_bass_kernel_spmd(nc, in_maps, list(range(NCORES)),
                               trace=_trace, tmpdir=_tmpdir)
    out = np.concatenate([res.results[c]["out"] for c in range(NCORES)], axis=0)
    kernel.last_result = res
    b2 = np.float32(np.asarray(fc2_b).reshape(-1)[0])
    return (out.reshape(-1, N, 1) + b2).astype(np.float32)


# revision 3
# speedup vs baseline: 1.1841x; 1.0013x over previous
"""Trainium2 Bass kernel for FNO1d (B=64, N=8192, W=64, MODES=16, 4 layers) — v2.

Data-parallel over batch: 8 elements/core, 4 pairs (2x64ch = 128 partitions).
rfft/irfft replaced by 16-mode DFT matmuls.

v2 optimizations over baseline:
- pointwise conv packed 4-elements-per-PE-pass via tile_position quadrants
  (off-diagonal quadrants swap element slots of the 2nd pair of each duo;
  compensated in the inverse-FT weight placement and final output mapping)
- inverse-FT as K=32 row-tiled matmuls, 2 pairs concurrent
- forward FT 4-way col-tiled (4 DFT chunks concurrently), partial sums
  summed with a stacked-identity matmul
- layer bias cb via ACT bias (fp32); layer-3 bias folded into fc1 bias
- fc1 2-way row-tiled; fc2 4-way col-tiled M=1; fc2 bias added on host
- mode-mix transposes on DVE (32x32 stream transposes) directly into SBUF
- fc head interleaved with layer 3; transposes split across sync/scalar DMA
  queues; FT accumulation runs in windows during the producing layer
"""
import sys
import numpy as np

sys.path.insert(0, "/opt/trn_rl_repo")

import concourse.bacc as bacc
import concourse.mybir as mybir
import concourse.tile as tile
from concourse.bass_utils import run_bass_kernel_spmd

F16 = mybir.dt.float16
F32 = mybir.dt.float32
GELU = mybir.ActivationFunctionType.Gelu

B, W, MODES, L = 64, 64, 16, 4
NCORES = 8
E = B // NCORES          # 8 elems per core
NP = E // 2              # 4 pairs
K2 = 2 * MODES           # 32 interleaved (cos,sin) rows

_cache = {}


def _build(N=8192):
    key = ("nc", N)
    if key in _cache:
        return _cache[key]
    NA = N // 128            # DFT chunks
    NGRP = N // 1024         # 1024-col groups per pair
    NW = NGRP // 2           # transpose windows (2048 cols each)
    APW = NA // NW           # a-chunks per window (16)
    assert APW == 16

    nc = bacc.Bacc("TRN2", target_bir_lowering=False, debug=False,
                   num_devices=NCORES)

    def din(name, shape, dt):
        return nc.dram_tensor(name, shape, dt, kind="ExternalInput").ap()

    xtib_d = din("xtib", [128, N], F16)   # rows 6p+(0..5)=(xA,tA,1,xB,tB,1); 32:64=IBr
    fb_d = din("fb", [128, NA * K2], F16)     # fb[p, a*32+kk] = Fb[128a+p, kk]
    ib_d = din("ib", [128, N], F16)           # rows 32p..32p+32 = IBr (x4)
    wm_d = din("wm", [64, L, MODES, 3, 64], F16)   # (i, l, k, {wr,wi,-wr}, o)
    cwt_d = din("cwt", [128, L * 128], F16)   # cw[l].T in all 4 quadrants
    w03_d = din("w03", [2, 64], F16)          # (w0; w1)
    w0f_d = din("w0f", [128, 512], F16)       # layer0 fused weights (u-blocks; mt at runtime)
    fc1w_d = din("fc1w", [128, 128], F16)     # fc1_w duplicated both halves
    fc2w_d = din("fc2w", [128, 1], F16)
    cbb_d = din("cbb", [128, 4], F32)         # per-layer ACT bias (l=3 unused)
    fc1b_d = din("fc1b", [128, 1], F32)       # fc1_b + fc1_w.T @ cb3
    i4_d = din("i4", [128, K2], F16)          # 4 stacked 32-identities
    idn_d = din("idn", [64, 64], F16)
    out_d = nc.dram_tensor("out", [E, N], F32, kind="ExternalOutput").ap()

    with tile.TileContext(nc) as tc:
        import contextlib
        with contextlib.ExitStack() as ctx:
            const = ctx.enter_context(tc.tile_pool(name="const", bufs=1))
            hpool = ctx.enter_context(tc.tile_pool(name="h", bufs=1))
            htp = ctx.enter_context(tc.tile_pool(name="htp", bufs=1))
            small = ctx.enter_context(tc.tile_pool(name="small", bufs=2))
            wmp = ctx.enter_context(tc.tile_pool(name="wmp", bufs=2))
            gpool = ctx.enter_context(tc.tile_pool(name="g", bufs=3))
            opool = ctx.enter_context(tc.tile_pool(name="o", bufs=2))
            ps = ctx.enter_context(tc.tile_pool(name="ps", bufs=1, space="PSUM"))

            # ---- constants (xtib first so transposes can start early) ----
            xtib = const.tile([128, N], F16, tag="xtib")
            nc.sync.dma_start(out=xtib[:], in_=xtib_d)
            fb = const.tile([128, NA * K2], F16, tag="fb")
            nc.sync.dma_start(out=fb[:], in_=fb_d)

            # persistent activations, one big tile [128, 4 pairs * N]
            h_all = hpool.tile([128, NP * N], F16, tag="h_all")
            # transposed view for FT: cols = (a, pair, slot*64+c)
            hta = htp.tile([128, NA * 512], F16, tag="hta")
            hta4 = hta[:].rearrange("p (a e c) -> p a e c", a=NA, e=NP)

            # layer-0 shortcut: transpose the 24 xt rows (+8 zeros) into hta
            xtt = hta[:, 0:NA * K2].rearrange("p (a c) -> p a c", a=NA)
            nc.sync.dma_start(out=xtt[:, :, 0:K2],
                              in_=xtib[0:K2, :], transpose=True)

            ib = const.tile([128, N], F16, tag="ib")
            nc.sync.dma_start(out=ib[:], in_=ib_d)
            cwt = const.tile([128, L * 128], F16, tag="cwt")
            nc.sync.dma_start(out=cwt[:], in_=cwt_d)
            w03 = const.tile([2, 64], F16, tag="w03")
            nc.sync.dma_start(out=w03[:], in_=w03_d)
            w0f = const.tile([128, 512], F16, tag="w0f")
            nc.sync.dma_start(out=w0f[:], in_=w0f_d)
            fc1w = const.tile([128, 128], F16, tag="fc1w")
            nc.sync.dma_start(out=fc1w[:], in_=fc1w_d)
            fc2w = const.tile([128, 1], F16, tag="fc2w")
            nc.sync.dma_start(out=fc2w[:], in_=fc2w_d)
            cbb = const.tile([128, 4], F32, tag="cbb")
            nc.sync.dma_start(out=cbb[:], in_=cbb_d)
            fc1b = const.tile([128, 1], F32, tag="fc1b")
            nc.sync.dma_start(out=fc1b[:], in_=fc1b_d)
            i4 = const.tile([128, K2], F16, tag="i4")
            nc.sync.dma_start(out=i4[:], in_=i4_d)
            idn = const.tile([64, 64], F16, tag="idn")
            nc.sync.dma_start(out=idn[:], in_=idn_d)

            wml = [None] * L
            wml[0] = wmp.tile([64, MODES * 3 * 64], F16, tag="wml", name="wml0")
            nc.sync.dma_start(out=wml[0][:],
                              in_=wm_d[:, 0].rearrange("p b c d -> p (b c d)"))

            def mix_chain(l, ftsb):
                """ftsb [32, 512] f16 (kk x (pair,slot,c)) -> mt4 [128,128] f16.

                mt4 rows 32p..32p+32 = kk, cols 64*tslot..+64 = o for pair p.
                tslot crossing for pairs 1,3 on quadrant layers (l>=1).
                """
                fti = small.tile([64, 256], F16, tag="fti")
                for hh in range(2):
                    for e in range(8):
                        nc.vector.transpose(
                            fti[32 * hh:32 * hh + 32, 32 * e:32 * e + 32],
                            ftsb[0:K2, 64 * e + 32 * hh:64 * e + 32 * hh + 32])
                mmp = ps.tile([64, 256], F32, tag="aux", name=f"mm{l}")
                for k in range(MODES):
                    wb = (k * 3) * 64
                    wr = wml[l][:, wb:wb + 64]
                    wi = wml[l][:, wb + 64:wb + 128]
                    nwr = wml[l][:, wb + 128:wb + 192]
                    ftc = fti[:, 2 * k::K2]
                    fts = fti[:, 2 * k + 1::K2]
                    nc.tensor.matmul(mmp[:, 16 * k:16 * k + 8], wr, ftc,
                                     start=True, stop=False)
                    nc.tensor.matmul(mmp[:, 16 * k:16 * k + 8], wi, fts,
                                     start=False, stop=True)
                    nc.tensor.matmul(mmp[:, 16 * k + 8:16 * k + 16], wi, ftc,
                                     start=True, stop=False)
                    nc.tensor.matmul(mmp[:, 16 * k + 8:16 * k + 16], nwr, fts,
                                     start=False, stop=True)
                mmsb = small.tile([64, 256], F16, tag="mmsb")
                nc.vector.tensor_copy(mmsb[:], mmp[:])
                # mmsb[o, 8*kk + eidx]; build mode matrices via DVE transposes
                if l == 0:
                    # fused layer-0 weights: mt blocks into w0f rows 32:64
                    for p in range(NP):
                        for s in range(2):
                            eidx = 2 * p + s
                            for oh in range(2):
                                nc.vector.transpose(
                                    w0f[32:64, 128 * p + 64 * s + 32 * oh:
                                        128 * p + 64 * s + 32 * oh + 32],
                                    mmsb[32 * oh:32 * oh + 32, eidx::8])
                    return None
                mt4 = small.tile([128, 128], F16, tag="mt4")
                for p in range(NP):
                    for s in range(2):
                        eidx = 2 * p + s
                        tslot = (1 - s) if p % 2 == 1 else s
                        for oh in range(2):
                            nc.vector.transpose(
                                mt4[32 * p:32 * p + 32,
                                    64 * tslot + 32 * oh:64 * tslot + 32 * oh + 32],
                                mmsb[32 * oh:32 * oh + 32, eidx::8])
                return mt4

            # ---------------- layer 0 head: ft from x/t spectra -------------
            fxp = ps.tile([128, K2], F32, tag="aux", name="fxp")
            nc.vector.memset(fxp[:], 0)
            for t in range(NA // 4):
                for j in range(4):
                    a = 4 * t + j
                    nc.tensor.matmul(fxp[32 * j:32 * j + 32, :],
                                     fb[:, a * K2:(a + 1) * K2],
                                     hta[:, a * K2:(a + 1) * K2],
                                     start=False, stop=False,
                                     skip_group_check=True,
                                     tile_position=(0, 32 * j))
            fxsb = small.tile([128, K2], F16, tag="fxsb")
            nc.vector.tensor_copy(fxsb[:], fxp[:])
            fxps = ps.tile([K2, K2], F32, tag="aux", name="fxps")
            nc.tensor.matmul(fxps[:], i4[:], fxsb[:], start=True, stop=True)
            fxs2 = small.tile([K2, K2], F16, tag="fxsb2")
            nc.vector.tensor_copy(fxs2[:], fxps[:])
            # per slot-elem transpose of (x,t) spectra -> [2, 32]
            ftxp = ps.tile([2, 256], F16, tag="aux", name="ftxp")
            for p in range(NP):
                for s in range(2):
                    cc = 6 * p + 3 * s
                    nc.tensor.transpose(ftxp[0:2, 32 * (2 * p + s):32 * (2 * p + s) + 32],
                                        fxs2[:, cc:cc + 2], idn[0:K2, 0:K2])
            fxt = small.tile([2, 256], F16, tag="fxt")
            nc.vector.tensor_copy(fxt[:], ftxp[:])
            ft0 = ps.tile([K2, 512], F32, tag="aux", name="ft0")
            for e in range(8):
                nc.tensor.matmul(ft0[:, 64 * e:64 * e + 64],
                                 fxt[0:2, 32 * e:32 * e + 32],
                                 w03[0:2, 0:64], start=True, stop=True)
            ftsb0 = small.tile([K2, 512], F16, tag="ftsb")
            nc.vector.tensor_copy(ftsb0[:], ft0[:])

            # ---------------- FNO layers ----------------
            for l in range(L):
                if l > 0:
                    ftpsb = small.tile([128, 512], F16, tag="ftpsb")
                    nc.vector.tensor_copy(ftpsb[:], ftp[:])  # noqa: F821
                    ftt = ps.tile([K2, 512], F32, tag="aux", name=f"ftt{l}")
                    nc.tensor.matmul(ftt[:], i4[:], ftpsb[:], start=True, stop=True)
                    ftsb = small.tile([K2, 512], F16, tag="ftsb")
                    nc.vector.tensor_copy(ftsb[:], ftt[:])
                else:
                    ftsb = ftsb0
                mt4 = mix_chain(l, ftsb)
                if l + 1 < L:
                    wml[l + 1] = wmp.tile([64, MODES * 3 * 64], F16, tag="wml",
                                          name=f"wml{l + 1}")
                    nc.sync.dma_start(
                        out=wml[l + 1][:],
                        in_=wm_d[:, l + 1].rearrange("p b c d -> p (b c d)"))
                if l < L - 1:
                    ftp = ps.tile([128, 512], F32, tag="ftp", name=f"ftp{l}")
                    nc.vector.memset(ftp[:], 0)

                def ft_window(w, half=None):
                    # transposes for window w (reads h_all written this layer);
                    # half=0/1 transposes only 1024 cols (8 a-chunks) for the
                    # fine-grained layer tail
                    a0 = w * APW if half in (None, 0) else w * APW + APW // 2
                    na = APW if half is None else APW // 2
                    for p in range(NP):
                        eng = nc.sync if p % 2 == 0 else nc.scalar
                        eng.dma_start(
                            out=hta4[:, a0:a0 + na, p, :],
                            in_=h_all[:, p * N + a0 * 128:p * N + (a0 + na) * 128],
                            transpose=True)

                def ft_burst(w, first=False, last=False, half=None):
                    t0 = 0 if half in (None, 0) else APW // 8
                    tn = APW // 4 if half is None else APW // 8
                    for t in range(t0, t0 + tn):
                        for j in range(4):
                            a = w * APW + 4 * t + j
                            nc.tensor.matmul(
                                ftp[32 * j:32 * j + 32, :],
                                fb[:, a * K2:(a + 1) * K2],
                                hta[:, a * 512:(a + 1) * 512],
                                start=False, stop=False,
                                skip_group_check=True,
                                tile_position=(0, 32 * j))

                act_l = cbb[:, l:l + 1]
                bursted = set()
                for g in range(NGRP):
                    for duo in range(2):
                        p0, p1 = 2 * duo, 2 * duo + 1
                        zp0 = gpsum_tile(ps, f"zp_l{l}g{g}d{duo}a")
                        zp1 = gpsum_tile(ps, f"zp_l{l}g{g}d{duo}b")
                        SL = (slice(0, 512), slice(512, 1024))
                        CO = (g * 1024, g * 1024 + 512)
                        if l == 0:
                            # fused layer 0: one K=64 matmul per (pair, c2)
                            # rhs rows 0:24 xt-rows (zero-weighted cross-pair),
                            # rows 32:64 ib; weights w0f = [u-blocks; mt]
                            for p, zp in ((p0, zp0), (p1, zp1)):
                                for c2 in range(2):
                                    nc.tensor.matmul(
                                        zp[:, SL[c2]],
                                        w0f[0:64, 128 * p:128 * p + 128],
                                        xtib[0:64, CO[c2]:CO[c2] + 512],
                                        start=True, stop=True)
                        else:
                            # inverse FT first (start=True clears each bank);
                            # c2-runs back-to-back reuse the loaded weights
                            for p, zp in ((p0, zp0), (p1, zp1)):
                                for c2 in range(2):
                                    nc.tensor.matmul(
                                        zp[:, SL[c2]], mt4[32 * p:32 * p + 32, :],
                                        ib[32 * p:32 * p + 32, CO[c2]:CO[c2] + 512],
                                        start=True, stop=False, skip_group_check=True,
                                        tile_position=(32 * p, 0))
                            lb = l * 128
                            quads = (
                                (zp0, slice(0, 64), cwt[0:64, lb:lb + 64],
                                 p0, slice(0, 64), (0, 0), False),
                                (zp1, slice(64, 128), cwt[0:64, lb + 64:lb + 128],
                                 p1, slice(0, 64), (0, 64), False),
                                (zp0, slice(64, 128), cwt[64:128, lb + 64:lb + 128],
                                 p0, slice(64, 128), (64, 64), True),
                                (zp1, slice(0, 64), cwt[64:128, lb:lb + 64],
                                 p1, slice(64, 128), (64, 0), True),
                            )
                            for zp, osl, cwq, hp, hsl, tpos, stp in quads:
                                for c2 in range(2):
                                    nc.tensor.matmul(
                                        zp[osl, SL[c2]], cwq,
                                        h_all[hsl, hp * N + CO[c2]:hp * N + CO[c2] + 512],
                                        start=False, stop=(stp and c2 == 1),
                                        skip_group_check=True,
                                        tile_position=tpos)
                        for p, zp in ((p0, zp0), (p1, zp1)):
                            dst = h_all[:, p * N + g * 1024:p * N + (g + 1) * 1024]
                            if l < L - 1:
                                nc.scalar.activation(dst, zp[:], GELU, bias=act_l)
                            else:
                                nc.vector.tensor_copy(dst, zp[:])
                    if l < L - 1:
                        # issue transposes as soon as a window's groups done;
                        # the last window goes in 1024-col halves to shrink
                        # the layer tail
                        if g == NGRP - 2:
                            ft_window(NW - 1, half=0)
                        elif g == NGRP - 1:
                            ft_window(NW - 1, half=1)
                            ft_burst(NW - 1, half=0)
                        elif g % 2 == 1:
                            ft_window(g // 2)
                        # FT bursts trail the transposes by ~2 groups
                        if g >= 3 and g % 2 == 1 and (g - 3) // 2 < NW - 1:
                            w = (g - 3) // 2
                            ft_burst(w)
                            bursted.add(w)
                    if l == L - 1:
                        # interleave fc head chunk g (reads h_all groups <= g)
                        fc_chunk(nc, g, h_all, fc1w, fc2w, fc1b, out_d,
                                 gpool, opool, ps, N)
                if l < L - 1:
                    for w in range(NW - 1):
                        if w not in bursted:
                            ft_burst(w)
                    ft_burst(NW - 1, half=1)

    nc.compile()
    _cache[key] = nc
    return nc


def gpsum_tile(ps, name):
    return ps.tile([128, 1024], F32, tag="zp", name=name, bufs=3)


def fc_chunk(nc, g, h_all, fc1w, fc2w, fc1b, out_d, gpool, opool, ps, N):
    """fc1 -> gelu -> fc2 for 1024-col chunk g, all pairs."""
    for p in range(NP):
        gsb = []
        for j in range(2):
            gp = gpsum_tile(ps, f"gp_g{g}p{p}j{j}")
            for c2 in range(2):
                cols = g * 1024 + c2 * 512
                nc.tensor.matmul(
                    gp[:, c2 * 512:(c2 + 1) * 512],
                    fc1w[64 * j:64 * j + 64, :],
                    h_all[64 * j:64 * j + 64, p * N + cols:p * N + cols + 512],
                    start=True, stop=True, tile_position=(64 * j, 0))
            gs = gpool.tile([128, 1024], F16, tag=f"gsb{j}", name=f"gsb{g}{p}{j}")
            nc.scalar.activation(gs[:], gp[:], GELU, bias=fc1b[:])
            gsb.append(gs)
        op = ps.tile([128, 512], F32, tag="aux", name=f"op{g}{p}")
        nc.vector.memset(op[:], 0)
        for j in range(2):
            for c2 in range(2):
                r = 32 * (2 * j + c2)
                nc.tensor.matmul(op[r:r + 1, :], fc2w[:],
                                 gsb[j][:, c2 * 512:(c2 + 1) * 512],
                                 start=True, stop=True, tile_position=(0, r))
        osb = opool.tile([128, 512], F32, tag="osb", name=f"osb{g}{p}")
        nc.vector.tensor_copy(osb[:], op[:])
        for j in range(2):
            # pairs 1,3 end with slots swapped (odd number of quadrant layers)
            erow = 2 * p + (1 - j if p % 2 == 1 else j)
            for c2 in range(2):
                r = 32 * (2 * j + c2)
                cols = g * 1024 + c2 * 512
                nc.sync.dma_start(out=out_d[erow:erow + 1, cols:cols + 512],
                                  in_=osb[r:r + 1, :])


def _consts(N, fc0_w, fc0_b, sw_r, sw_i, cw, cb, fc1_w, fc1_b, fc2_w, fc2_b):
    f16 = np.float16
    NA = N // 128
    n = np.arange(N)
    k = np.arange(MODES)
    ang = 2.0 * np.pi * np.outer(n, k) / N            # [N, MODES]
    Fb = np.empty((N, K2), np.float32)
    Fb[:, 0::2] = np.cos(ang)
    Fb[:, 1::2] = np.sin(ang)
    fb = np.ascontiguousarray(
        Fb.reshape(NA, 128, K2).transpose(1, 0, 2)).reshape(128, NA * K2).astype(f16)

    alpha = np.where(k == 0, 1.0, 2.0)
    IBr = np.empty((K2, N), np.float32)
    IBr[0::2, :] = (alpha[:, None] * np.cos(ang.T)) / N
    IBr[1::2, :] = -2.0 * np.sin(ang.T) / N
    ib = np.zeros((128, N), np.float32)
    for p in range(NP):
        ib[32 * p:32 * p + K2] = IBr
    ib = ib.astype(f16)

    wm = np.empty((64, L, MODES, 3, 64), np.float32)
    for l in range(L):
        for kk in range(MODES):
            wm[:, l, kk, 0, :] = sw_r[l, :, :, kk]
            wm[:, l, kk, 1, :] = sw_i[l, :, :, kk]
            wm[:, l, kk, 2, :] = -sw_r[l, :, :, kk]
    wm = wm.astype(f16)

    cwt = np.zeros((128, L * 128), np.float32)
    for l in range(L):
        for qr in range(2):
            for qc in range(2):
                cwt[64 * qr:64 * qr + 64, l * 128 + 64 * qc:l * 128 + 64 * qc + 64] = cw[l].T

    cbb = np.zeros((128, 4), np.float32)
    for l in range(L - 1):
        cbl = cb[l].astype(np.float32).copy()
        if l == 0:
            cbl = cbl + sw_r[0, :, :, 0].T @ fc0_b
        cbb[0:64, l] = cbl
        cbb[64:128, l] = cbl

    w03 = fc0_w.astype(np.float32)                      # [2, 64]
    u = np.stack([cw[0] @ fc0_w[0], cw[0] @ fc0_w[1], cw[0] @ fc0_b], axis=0)
    w0f = np.zeros((128, 512), np.float32)
    for p in range(NP):
        w0f[6 * p:6 * p + 3, 128 * p:128 * p + 64] = u
        w0f[6 * p + 3:6 * p + 6, 128 * p + 64:128 * p + 128] = u

    fc1bp = fc1_b.astype(np.float32) + fc1_w.astype(np.float32).T @ cb[L - 1].astype(np.float32)

    i4c = np.zeros((128, K2), np.float32)
    for j in range(4):
        i4c[32 * j:32 * j + K2, :] = np.eye(K2)

    return dict(
        fb=fb, ib=ib, wm=wm,
        cwt=cwt.astype(f16), cbb=cbb, w03=w03.astype(f16),
        w0f=w0f.astype(f16),
        fc1w=np.concatenate([fc1_w, fc1_w], axis=0).astype(f16),
        fc2w=fc2_w.astype(f16),
        fc1b=fc1bp.reshape(128, 1),
        i4=i4c.astype(f16),
        idn=np.eye(64, dtype=f16),
    )


def make_xtib(x, t, core, N, ibr):
    xt = np.zeros((128, N), np.float16)
    for p in range(NP):
        eA, eB = core * E + 2 * p, core * E + 2 * p + 1
        xt[6 * p + 0] = x[eA]
        xt[6 * p + 1] = t[eA]
        xt[6 * p + 2] = 1.0
        xt[6 * p + 3] = x[eB]
        xt[6 * p + 4] = t[eB]
        xt[6 * p + 5] = 1.0
    xt[32:64] = ibr
    return xt


def kernel(x, t, fc0_w, fc0_b, sw_r, sw_i, cw, cb, fc1_w, fc1_b, fc2_w, fc2_b,
           _trace=False, _tmpdir=None):
    N = np.asarray(x).shape[1]
    nc = _build(N)
    consts = _consts(N, np.asarray(fc0_w), np.asarray(fc0_b), np.asarray(sw_r),
                     np.asarray(sw_i), np.asarray(cw), np.asarray(cb),
                     np.asarray(fc1_w), np.asarray(fc1_b), np.asarray(fc2_w),
                     np.asarray(fc2_b))
    x = np.asarray(x, np.float32).reshape(-1, N).astype(np.float16)
    t = np.asarray(t, np.float32).reshape(-1, N).astype(np.float16)
    in_maps = []
    for c in range(NCORES):
        m = dict(consts)
        m["xtib"] = make_xtib(x, t, c, N, consts["ib"][0:K2])
        in_maps.append(m)
    res = run_bass_kernel_spmd(nc, in_maps, list(range(NCORES)),
                               trace=_trace, tmpdir=_tmpdir)
    out = np.concatenate([res.results[c]["out"] for c in range(NCORES)], axis=0)
    kernel.last_result = res
    b2 = np.float32(np.asarray(fc2_b).reshape(-1)[0])
    return (out.reshape(-1, N, 1) + b2).astype(np.float32)
